# revision 32
# baseline (speedup 1.0000x reference)
"""MultiHeadSectionAttentionImputer on 8 TRN2 NeuronCores (Bass/Tile).

Sharding: 2 head-groups x 4 key-shards. Core c handles heads
[6*(c//4), 6*(c//4)+6) and exist-keys [1536*(c%4), 1536*(c%4)+1536).
Each core:
  - projects its key shard to K,V (K = X_e @ Wk; V = X_e @ Wv with an
    appended ones column), its 6 heads only
  - projects the full missing set to Q for its 6 heads (Wq,bq pre-scaled
    by 1/sqrt(d_k) on host; bk dropped - it only shifts scores by a
    per-query constant, softmax-invariant and consistent across shards)
  - computes scoresT[key, query] per head with a fused 128-deep
    contraction: d' = [q-dims(64) | cooc-bias-dims(64)] so one matmul
    yields q.k/sqrt(dk) + mb.eb
  - exp() without max subtraction (scores bounded ~<60, safe in fp32)
  - attn @ [V | 1] accumulated over the 12 key chunks -> partial
    numerators (64 cols) + denominator per query
Host combines partials across the 4 key-shards of each head group
(exact softmax over all 6144 keys), adds bv, scatters into ehr.

Matmul inputs are fp16 (psum accumulates fp32); the attention weights
are bf16 (exp output needs fp32-like range; no max subtraction).

Perf notes (209.9us -> ~172.2us on HW):
- exp() was the co-bottleneck with PE (144 ACTIVATEs ~ 162us on
  ScalarE alone). It is now split: 8/12 key-chunks exact on ScalarE,
  4/12 on VectorE via a Schraudolph bit-trick (bf16 bits =
  round(x*128/ln2 + B) through tensor_scalar f32->uint16; ~2% rms
  multiplicative noise, the common bias cancels in softmax). Chunk 11
  on the DVE matters: the last tile of each half gates the AV group.
- Q and K projections run in fp8 e4m3 with DoubleRow (K=256 per
  matmul, halves their PE stream cycles and input DMA bytes). Weights
  are host-prescaled by 8 into fp8 normal range; the psum rescale
  rides the existing bias-add/copy (x1/64 resp x1/8). V stays fp16
  (its values land directly in the output).
- PSUM: 3 score bufs [128,1024] (6 banks) decouple PE from exp-engine
  latency jitter + 2 shared proj/av bufs (2 banks).
- AV outputs copied/DMAed in 4-query-chunk groups; input DMAs are
  need-ordered across the 3 DMA rings (measured ~108/69/126 GB/s under
  contention); unit pumping skips the first 2 slots after each half
  boundary so queued AV groups don't head-of-line block PE on the
  previous half's last exp tile.
"""

import os
import sys
import numpy as np
from contextlib import ExitStack

sys.path.insert(0, "/opt/trn_rl_repo")

# problem constants (hardcoded; kernel.py must be self-contained)
H = 12          # total heads
DK = 64         # head dim
E = 768         # embed dim
TOTAL = H * DK  # 768
M = 2048        # missing sections
N = 6144        # existing sections
CORES = 8
HGROUPS = 2     # head groups (cores 0-3 -> heads 0-5, cores 4-7 -> 6-11)
NSHARDS = 4
HH = H // HGROUPS        # 6 heads per core
PP = HH // 2             # 3 head pairs per core
TT = HH * DK             # 384 projection cols per core
NLOC = N // NSHARDS      # 1536 keys per core
EC = E // 128            # 6 contraction chunks
NI = NLOC // 128         # 12 key chunks per core
MI = M // 128            # 16 query chunks

_CACHE = {}
LAST_EXEC_NS = None
LAST_TRACE_DIR = None


def _build():
    import concourse.bass as bass
    import concourse.tile as tile
    from concourse import bacc, mybir
    from collections import deque

    F32 = mybir.dt.float32
    FP16 = mybir.dt.float16
    BF16 = mybir.dt.bfloat16
    U16 = mybir.dt.uint16
    FP8 = mybir.dt.float8e4
    DoubleRow = mybir.MatmulPerfMode.DoubleRow
    Exp = mybir.ActivationFunctionType.Exp
    Mult = mybir.AluOpType.mult
    Add = mybir.AluOpType.add
    # Schraudolph exp: bf16 bits = round(x * 128/ln2 + B). B centered for
    # min variation; the shared multiplicative bias cancels in softmax.
    EXP_A = 128.0 / float(np.log(2.0))
    EXP_B = 16246.8
    DVE_NI = (2, 5, 8, 11)  # key-chunks whose exp runs on VectorE

    nc = bacc.Bacc("TRN2", target_bir_lowering=False, debug=False)

    # ---- I/O (layouts chosen so every DMA is contiguous) ----
    xt_m = nc.dram_tensor("xt_m", [128, 4, EC // 2, 2, 512], FP8,
                          kind="ExternalInput").ap()
    mbt = nc.dram_tensor("mbt", [HH * DK, M], FP16, kind="ExternalInput").ap()
    xt_e = nc.dram_tensor("xt_e", [128, 3, EC, 512], FP16, kind="ExternalInput").ap()
    xt_e8 = nc.dram_tensor("xt_e8", [128, 3, EC // 2, 2, 512], FP8,
                           kind="ExternalInput").ap()
    ebt = nc.dram_tensor("ebt", [HH * DK, NLOC], FP16, kind="ExternalInput").ap()
    wq = nc.dram_tensor("wq", [128, PP, EC // 2, 2, 128], FP8,
                        kind="ExternalInput").ap()
    wk = nc.dram_tensor("wk", [128, PP, EC // 2, 2, 128], FP8,
                        kind="ExternalInput").ap()
    wv = nc.dram_tensor("wv", [128, EC, TT], FP16, kind="ExternalInput").ap()
    bq = nc.dram_tensor("bq", [128, PP], F32, kind="ExternalInput").ap()
    out_p = nc.dram_tensor("out_p", [HH, M, DK + 1], F32, kind="ExternalOutput").ap()

    with tile.TileContext(nc) as tc, ExitStack() as ctx:
        persist = ctx.enter_context(tc.tile_pool(name="persist", bufs=1))
        qpt_pool = ctx.enter_context(tc.tile_pool(name="qpt", bufs=5))
        attn_pool = ctx.enter_context(tc.tile_pool(name="attn", bufs=24))
        osb_pool = ctx.enter_context(tc.tile_pool(name="osb", bufs=6))
        # PSUM: scores 3 x [128,1024] (6 banks) + one shared pool for the
        # proj [128,512] and av [128,4,65] accumulators (2 banks) = 8.
        # 3 score bufs decouple PE from ACT/DVE exp-latency jitter.
        work_ps = ctx.enter_context(tc.tile_pool(name="work_ps", bufs=2, space="PSUM"))
        sc_ps = ctx.enter_context(tc.tile_pool(name="sc_ps", bufs=3, space="PSUM"))
        proj_ps = work_ps
        av_ps = work_ps

        # K'T per head [128, NLOC]: rows = k-dims | eb-dims (parity layout:
        # even head k at partitions 0:64, odd head k at 64:128 - avoids any
        # cross-partition copies; scores only need a consistent d' order)
        kpt = [persist.tile([128, NLOC], FP16, tag=f"kpt{h}", name=f"kpt{h}")
               for h in range(HH)]
        # V per key chunk [128, HH, DK+1] bf16, ones col at [., ., DK]
        vsb = [persist.tile([128, HH, DK + 1], BF16, tag=f"v{ni}", name=f"v{ni}")
               for ni in range(NI)]
        bq_sb = persist.tile([128, PP], F32, tag="bq")
        wk_big = persist.tile([128, PP, EC // 2, 2, 128], FP8, tag="wk")
        wq_big = persist.tile([128, PP, EC // 2, 2, 128], FP8, tag="wq")
        wv_big = persist.tile([128, EC, TT], FP16, tag="wv")
        xte_big = persist.tile([128, 3, EC, 512], FP16, tag="xte")
        xte8_big = persist.tile([128, 3, EC // 2, 2, 512], FP8, tag="xte8")
        xtm_big = persist.tile([128, 4, EC // 2, 2, 512], FP8, tag="xtm")
        q0_0 = qpt_pool.tile([128, M], FP16, tag="qpt", name="qpt0")
        q1_0 = qpt_pool.tile([128, M], FP16, tag="qpt", name="qpt1")

        # input DMAs: need-ordered across the two HWDGE rings (sync=SP,
        # scalar=Act) plus gpsimd SWDGE rows.
        # measured ring rates under contention: sync(SP)~108GB/s,
        # scalar(Act)~69GB/s, gpsimd(SWDGE)~126GB/s. Critical chains are
        # spread by deadline: first-scores needs wk8+xte8_0+ebt_h0 (SP),
        # wq8 (Act), mbt_h0+xtm8_0/1 (SW) by ~15us; kt thirds 1/2 and the
        # half-1 quarters follow; bulk fp16 xte/wv (V-proj, ~35us) last.
        nc.scalar.dma_start(bq_sb[:], bq)
        nc.sync.dma_start(wk_big[:, 0], wk[:, 0])
        nc.scalar.dma_start(q0_0[64:128, :], mbt[0:DK, :])
        nc.gpsimd.dma_start(xtm_big[:, 0], xt_m[:, 0])
        nc.sync.dma_start(xte8_big[:, 0], xt_e8[:, 0])
        nc.scalar.dma_start(wq_big[:, 0], wq[:, 0])
        nc.gpsimd.dma_start(xtm_big[:, 1], xt_m[:, 1])
        nc.sync.dma_start(kpt[0][64:128, :], ebt[0:DK, :])
        nc.scalar.dma_start(xte8_big[:, 2], xt_e8[:, 2])
        nc.sync.dma_start(xte8_big[:, 1], xt_e8[:, 1])
        nc.gpsimd.dma_start(kpt[1][0:64, :], ebt[DK:2 * DK, :])
        nc.gpsimd.dma_start(q1_0[0:64, :], mbt[DK:2 * DK, :])
        # remainder
        nc.scalar.dma_start(xtm_big[:, 3], xt_m[:, 3])
        nc.gpsimd.dma_start(xtm_big[:, 2], xt_m[:, 2])
        nc.scalar.dma_start(wk_big[:, 1:PP], wk[:, 1:PP])
        nc.gpsimd.dma_start(wv_big[:], wv[:])
        nc.sync.dma_start(xte_big[:, 0], xt_e[:, 0])
        nc.sync.dma_start(xte_big[:, 1], xt_e[:, 1])
        nc.gpsimd.dma_start(xte_big[:, 2], xt_e[:, 2])
        nc.sync.dma_start(wq_big[:, 1:PP], wq[:, 1:PP])

        # PE warm-up: ~10 dummy matmuls on zeroed tiles run during the
        # initial DMA wait (PE is otherwise idle until ~12.6us) so the
        # HAM clock-gate flips to 8/8 before the real ramp matmuls --
        # those otherwise run at 1.2GHz for the first ~3.4us-busy window
        # (12.2us throttle_active measured, ~5.6us extra MM latency).
        warm_w = persist.tile([128, 128], FP16, tag="warm_w")
        warm_x = persist.tile([128, 512], FP16, tag="warm_x")
        nc.vector.memset(warm_w[:], 0.0)
        nc.vector.memset(warm_x[:], 0.0)
        wps = proj_ps.tile([128, 512], F32, tag="work", name="warm_ps")
        for wi in range(10):
            nc.tensor.matmul(wps[:], lhsT=warm_w[:], rhs=warm_x[:],
                             start=(wi == 0), stop=(wi == 9))

        def emit_scores_mj(ps, h, qt, ni, half, mj):
            mo = half * 1024
            nc.tensor.matmul(
                ps[:, mj * 512:(mj + 1) * 512],
                lhsT=kpt[h][:, ni * 128:(ni + 1) * 128],
                rhs=qt[:, mo + mj * 512:mo + (mj + 1) * 512],
                start=True, stop=True)

        def emit_exp(ps, h, ni, half, at):
            mo = half * 1024
            if ni in DVE_NI:
                nc.vector.tensor_scalar(
                    at[:, mo:mo + 1024].bitcast(U16), ps[:],
                    EXP_A, EXP_B, op0=Mult, op1=Add)
            else:
                nc.scalar.activation(at[:, mo:mo + 1024], ps[:], Exp)

        def emit_scores_exp_half(h, qt, ni, half, at, pre=None):
            """scoresT half [128 keys, 1024 queries] + exp into attnT.
            exp runs on ScalarE (exact) or VectorE (Schraudolph bits)
            depending on the key-chunk, so the two engines drain scores
            psum concurrently."""
            ps = pre if pre is not None else sc_ps.tile(
                [128, 1024], F32, tag="sc", name="sc_ps_t")
            for mj in ((1,) if pre is not None else (0, 1)):
                emit_scores_mj(ps, h, qt, ni, half, mj)
            emit_exp(ps, h, ni, half, at)

        def emit_av(h, attns, g4):
            """out chunks for 4 query-blocks mi in [4*g4, 4*g4+4); the
            bf16 attnT chunk is the stationary operand (fast weight
            load). One grouped copy + DMA per 4 blocks."""
            ps = av_ps.tile([128, 4, DK + 1], F32, tag="work", name="av_ps_t")
            for j in range(4):
                mi = 4 * g4 + j
                for ni in range(NI):
                    nc.tensor.matmul(
                        ps[:, j, :], lhsT=attns[ni][:, mi * 128:(mi + 1) * 128],
                        rhs=vsb[ni][:, h, :],
                        start=(ni == 0), stop=(ni == NI - 1))
            ot = osb_pool.tile([128, 4, DK + 1], F32, tag="osb", name="osb_t")
            nc.vector.tensor_copy(ot[:], ps[:])
            m0 = 4 * g4 * 128
            nc.sync.dma_start(
                out_p[h, m0:m0 + 512, :].rearrange("(j p) d -> p j d", p=128),
                ot[:])

        # ---- unit-queue scheduler ----
        units = deque()
        qts = {0: q0_0, 1: q1_0}
        pieces = {0: set()}  # pair -> done piece ids (k0..k2, q0..q3)

        def emit_qt_quarter(p, mh, q0, q1):
            ps = proj_ps.tile([128, 512], F32, tag="work", name="proj_qt")
            for e2 in range(EC // 2):
                nc.tensor.matmul(ps[:], lhsT=wq_big[:, p, e2],
                                 rhs=xtm_big[:, mh, e2],
                                 start=(e2 == 0), stop=(e2 == EC // 2 - 1),
                                 perf_mode=DoubleRow)
            mo = mh * 512
            nc.vector.tensor_scalar(
                q0[0:64, mo:mo + 512], ps[0:64, :], 0.015625,
                bq_sb[0:64, p:p + 1], op0=Mult, op1=Add)
            nc.vector.tensor_scalar(
                q1[64:128, mo:mo + 512], ps[64:128, :], 0.015625,
                bq_sb[64:128, p:p + 1], op0=Mult, op1=Add)

        def qt_unit(p, mh):
            def f():
                pieces.setdefault(p, set()).add(f"q{mh}")
                q0, q1 = qts.get(2 * p), qts.get(2 * p + 1)
                if q0 is None:
                    q0 = qpt_pool.tile([128, M], FP16, tag="qpt", name=f"qpt{2*p}")
                    q1 = qpt_pool.tile([128, M], FP16, tag="qpt", name=f"qpt{2*p+1}")
                    h0, h1 = 2 * p, 2 * p + 1
                    nc.gpsimd.dma_start(q0[64:128, :], mbt[h0 * DK:(h0 + 1) * DK, :])
                    nc.gpsimd.dma_start(q1[0:64, :], mbt[h1 * DK:(h1 + 1) * DK, :])
                    qts[2 * p], qts[2 * p + 1] = q0, q1
                emit_qt_quarter(p, mh, q0, q1)
            return (1.35, f)

        def emit_kt_third(p, t):
            h0, h1 = 2 * p, 2 * p + 1
            lo = t * 512
            ps = proj_ps.tile([128, 512], F32, tag="work", name="proj_kt")
            for e2 in range(EC // 2):
                nc.tensor.matmul(ps[:], lhsT=wk_big[:, p, e2],
                                 rhs=xte8_big[:, t, e2],
                                 start=(e2 == 0), stop=(e2 == EC // 2 - 1),
                                 perf_mode=DoubleRow)
            nc.vector.tensor_scalar(kpt[h0][0:64, lo:lo + 512], ps[0:64, :],
                                    0.125, None, op0=Mult)
            nc.vector.tensor_scalar(kpt[h1][64:128, lo:lo + 512], ps[64:128, :],
                                    0.125, None, op0=Mult)
            if t == 0 and p > 0:
                nc.gpsimd.dma_start(kpt[h0][64:128, :], ebt[h0 * DK:(h0 + 1) * DK, :])
                nc.gpsimd.dma_start(kpt[h1][0:64, :], ebt[h1 * DK:(h1 + 1) * DK, :])

        def kt_unit(p, t):
            def f():
                pieces.setdefault(p, set()).add(f"k{t}")
                emit_kt_third(p, t)
            return (1.35, f)

        def v_unit(ni):
            def f():
                ps = proj_ps.tile([128, TT], F32, tag="work", name="proj_v")
                t, off = divmod(ni, 4)
                for ec in range(EC):
                    nc.tensor.matmul(
                        ps[:], lhsT=xte_big[:, t, ec, off * 128:(off + 1) * 128],
                        rhs=wv_big[:, ec, :], start=(ec == 0), stop=(ec == EC - 1))
                nc.vector.tensor_copy(
                    vsb[ni][:, :, 0:DK], ps[:].rearrange("p (h d) -> p h d", d=DK))
                nc.vector.memset(vsb[ni][:, :, DK], 1.0)
            return (1.0, f)

        def av_unit(h, attns, g4):
            def f():
                emit_av(h, attns, g4)
            return (1.6, f)

        def pump(budget):
            while units and budget > 0:
                c, f = units.popleft()
                f()
                budget -= c

        # minimal head-0 critical path up front: kt third0 + qt q0/q1
        emit_kt_third(0, 0)
        pieces[0].add("k0")
        qt_unit(0, 0)[1]()
        qt_unit(0, 1)[1]()
        units.append(kt_unit(0, 2))
        units.append(kt_unit(0, 1))
        units.append(qt_unit(0, 2))
        units.append(qt_unit(0, 3))
        for ni in range(NI):
            units.append(v_unit(ni))

        def need(p, ni, half):
            req = {f"k{ni // 4}", f"q{2 * half}", f"q{2 * half + 1}"}
            while not req <= pieces.get(p, set()):
                c, f = units.popleft()
                f()

        slot = 0
        for h in range(HH):
            p = h // 2
            if h % 2 == 1 and p + 1 <= PP - 1:
                # next pair's projections jump the queue (front) so the
                # even-head boundary never force-drains a big batch
                for mh in range(3, -1, -1):
                    units.appendleft(qt_unit(p + 1, mh))
                for t in range(2, -1, -1):
                    units.appendleft(kt_unit(p + 1, t))
            attns = [attn_pool.tile([128, M], BF16, tag="attn",
                                    name=f"attn_{h}_{ni}") for ni in range(NI)]
            for half in range(2):
                # h0-half0: key-third 2 (Act ring) lands before third 1
                # (SP ring) -- consume chunks in DMA-arrival order
                ni_order = ([0, 1, 2, 3, 8, 9, 10, 11, 4, 5, 6, 7]
                            if (h == 0 and half == 0) else range(NI))
                for ni in ni_order:
                    need(p, ni, half)
                    emit_scores_exp_half(h, qts[h], ni, half, attns[ni])
                    # no pumping on the first 2 chunks after a half
                    # boundary: the queued av group would head-of-line
                    # block PE on the previous half's last exp tile
                    if slot < 12:
                        b = 0.0
                    elif slot < 24:
                        b = 2.5
                    elif ni < 2:
                        b = 0.0
                    else:
                        b = 1.1
                    pump(b)
                    slot += 1
                # av groups for mi chunks covered by this half can go
                # into the queue now (g4<2 reads attnT cols 0:1024 only)
                gs = range(2) if half == 0 else range(2, 4)
                for g4 in gs:
                    units.append(av_unit(h, attns, g4))
            qts[h] = None  # release the qpt slot
        while units:
            c, f = units.popleft()
            f()

    nc.compile()
    return nc


def _get_nc():
    if "nc" not in _CACHE:
        _CACHE["nc"] = _build()
    return _CACHE["nc"]


def kernel(**inputs):
    global LAST_EXEC_NS, LAST_TRACE_DIR
    from concourse.bass_utils import run_bass_kernel_spmd

    ehr = np.asarray(inputs["ehr_embeddings"], dtype=np.float32)
    mi = np.asarray(inputs["missing_indices"]).astype(np.int64)
    ei = np.asarray(inputs["exist_indices"]).astype(np.int64)
    Wq = np.asarray(inputs["Wq"], dtype=np.float32)
    Wk = np.asarray(inputs["Wk"], dtype=np.float32)
    Wv = np.asarray(inputs["Wv"], dtype=np.float32)
    bq = np.asarray(inputs["bq"], dtype=np.float32)
    bv = np.asarray(inputs["bv"], dtype=np.float32)
    cooc = np.asarray(inputs["cooc_bias"], dtype=np.float32)

    scale = 1.0 / np.sqrt(np.float32(DK))

    def fold(a):  # [E, F] -> [128, EC, F]
        return a.reshape(EC, 128, a.shape[1]).transpose(1, 0, 2)

    def wfold(a):  # [E, TT] -> [128, PP, EC, 128] (pair-col major)
        return np.ascontiguousarray(
            fold(a).reshape(128, EC, PP, 128).transpose(0, 2, 1, 3))

    import ml_dtypes
    F8NP = ml_dtypes.float8_e4m3

    missing_emb = ehr[mi]                       # [M, E]
    # Q-proj operands in fp8 e4m3 (DoubleRow): Wq pre-scaled by 8 so its
    # values sit in fp8 normal range; the kernel rescales psum by 1/64.
    xt_m = np.ascontiguousarray(
        fold(missing_emb.T)
        .reshape(128, EC, 4, 512).transpose(0, 2, 1, 3)
        .reshape(128, 4, EC // 2, 2, 512)).astype(F8NP)
    wq_all = (Wq * 8.0).astype(np.float32)
    wk_all = (Wk * 8.0).astype(np.float32)
    wv_all = Wv.astype(np.float16)
    mbt_all = cooc[:, mi, :].transpose(0, 2, 1).reshape(H * DK, M).astype(np.float16)
    bq_all = (bq * scale).astype(np.float32)

    in_maps = []
    for c in range(CORES):
        hg, ns = c // NSHARDS, c % NSHARDS
        hsl = slice(hg * TT, (hg + 1) * TT)
        eic = ei[ns * NLOC:(ns + 1) * NLOC]
        xte_f32 = fold(ehr[eic].T)                   # [128, EC, NLOC]
        xte_t = xte_f32.reshape(128, EC, 3, 512).transpose(0, 2, 1, 3)
        xt_e = np.ascontiguousarray(xte_t).astype(np.float16)
        xt_e8 = np.ascontiguousarray(
            xte_t.reshape(128, 3, EC // 2, 2, 512)).astype(F8NP)
        ebt = np.ascontiguousarray(
            cooc[hg * HH:(hg + 1) * HH, eic, :].transpose(0, 2, 1)
            .reshape(HH * DK, NLOC).astype(np.float16))
        in_maps.append({
            "xt_m": xt_m,
            "mbt": np.ascontiguousarray(mbt_all[hsl]),
            "xt_e": xt_e, "xt_e8": xt_e8, "ebt": ebt,
            "wq": np.ascontiguousarray(
                wfold(wq_all[:, hsl]).reshape(128, PP, EC // 2, 2, 128)
            ).astype(F8NP),
            "wk": np.ascontiguousarray(
                wfold(wk_all[:, hsl]).reshape(128, PP, EC // 2, 2, 128)
            ).astype(F8NP),
            "wv": np.ascontiguousarray(fold(wv_all[:, hsl])),
            "bq": np.ascontiguousarray(bq_all[hsl].reshape(PP, 128).T),
        })

    nc = _get_nc()
    kwargs = {}
    if os.environ.get("KERNEL_TRACE") == "1":
        import tempfile
        LAST_TRACE_DIR = tempfile.mkdtemp(prefix="kern_trace_")
        kwargs = {"trace": True, "tmpdir": LAST_TRACE_DIR}
        try:
            import ntff_shim
            ntff_shim.install()
        except ImportError:
            pass
    res = run_bass_kernel_spmd(nc, in_maps, list(range(CORES)), **kwargs)
    LAST_EXEC_NS = res.exec_time_ns

    # ---- host combine (exact softmax across the 4 key shards) ----
    num = np.zeros((H, M, DK), dtype=np.float64)
    den = np.zeros((H, M), dtype=np.float64)
    for c in range(CORES):
        hg = c // NSHARDS
        op = res.results[c]["out_p"].astype(np.float64)  # [HH, M, DK+1]
        num[hg * HH:(hg + 1) * HH] += op[:, :, :DK]
        den[hg * HH:(hg + 1) * HH] += op[:, :, DK]
    out = num / den[:, :, None]                          # [H, M, DK]
    out = out.transpose(1, 0, 2).reshape(M, TOTAL) + bv.astype(np.float64)
    result = ehr.copy()
    result[mi] = out.astype(np.float32)
    return result



# revision 33
# speedup vs baseline: 1.0051x; 1.0051x over previous
"""MultiHeadSectionAttentionImputer on 8 TRN2 NeuronCores (Bass/Tile).

Sharding: 2 head-groups x 4 key-shards. Core c handles heads
[6*(c//4), 6*(c//4)+6) and exist-keys [1536*(c%4), 1536*(c%4)+1536).
Each core:
  - projects its key shard to K,V (K = X_e @ Wk; V = X_e @ Wv with an
    appended ones column), its 6 heads only
  - projects the full missing set to Q for its 6 heads (Wq,bq pre-scaled
    by 1/sqrt(d_k) on host; bk dropped - it only shifts scores by a
    per-query constant, softmax-invariant and consistent across shards)
  - computes scoresT[key, query] per head with a fused 128-deep
    contraction: d' = [q-dims(64) | cooc-bias-dims(64)] so one matmul
    yields q.k/sqrt(dk) + mb.eb
  - exp() without max subtraction (scores bounded ~<60, safe in fp32)
  - attn @ [V | 1] accumulated over the 12 key chunks -> partial
    numerators (64 cols) + denominator per query
Host combines partials across the 4 key-shards of each head group
(exact softmax over all 6144 keys), adds bv, scatters into ehr.

Matmul inputs are fp16 (psum accumulates fp32); the attention weights
are bf16 (exp output needs fp32-like range; no max subtraction).

Perf notes (209.9us -> ~172.2us on HW):
- exp() was the co-bottleneck with PE (144 ACTIVATEs ~ 162us on
  ScalarE alone). It is now split: 8/12 key-chunks exact on ScalarE,
  4/12 on VectorE via a Schraudolph bit-trick (bf16 bits =
  round(x*128/ln2 + B) through tensor_scalar f32->uint16; ~2% rms
  multiplicative noise, the common bias cancels in softmax). Chunk 11
  on the DVE matters: the last tile of each half gates the AV group.
- Q and K projections run in fp8 e4m3 with DoubleRow (K=256 per
  matmul, halves their PE stream cycles and input DMA bytes). Weights
  are host-prescaled by 8 into fp8 normal range; the psum rescale
  rides the existing bias-add/copy (x1/64 resp x1/8). V stays fp16
  (its values land directly in the output).
- PSUM: 3 score bufs [128,1024] (6 banks) decouple PE from exp-engine
  latency jitter + 2 shared proj/av bufs (2 banks).
- AV outputs copied/DMAed in 4-query-chunk groups; input DMAs are
  need-ordered across the 3 DMA rings (measured ~108/69/126 GB/s under
  contention); unit pumping skips the first 2 slots after each half
  boundary so queued AV groups don't head-of-line block PE on the
  previous half's last exp tile.
"""

import os
import sys
import numpy as np
from contextlib import ExitStack

sys.path.insert(0, "/opt/trn_rl_repo")

# problem constants (hardcoded; kernel.py must be self-contained)
H = 12          # total heads
DK = 64         # head dim
E = 768         # embed dim
TOTAL = H * DK  # 768
M = 2048        # missing sections
N = 6144        # existing sections
CORES = 8
HGROUPS = 2     # head groups (cores 0-3 -> heads 0-5, cores 4-7 -> 6-11)
NSHARDS = 4
HH = H // HGROUPS        # 6 heads per core
PP = HH // 2             # 3 head pairs per core
TT = HH * DK             # 384 projection cols per core
NLOC = N // NSHARDS      # 1536 keys per core
EC = E // 128            # 6 contraction chunks
NI = NLOC // 128         # 12 key chunks per core
MI = M // 128            # 16 query chunks

_CACHE = {}
LAST_EXEC_NS = None
LAST_TRACE_DIR = None


def _build():
    import concourse.bass as bass
    import concourse.tile as tile
    from concourse import bacc, mybir
    from collections import deque

    F32 = mybir.dt.float32
    FP16 = mybir.dt.float16
    BF16 = mybir.dt.bfloat16
    U16 = mybir.dt.uint16
    FP8 = mybir.dt.float8e4
    DoubleRow = mybir.MatmulPerfMode.DoubleRow
    Exp = mybir.ActivationFunctionType.Exp
    Mult = mybir.AluOpType.mult
    Add = mybir.AluOpType.add
    # Schraudolph exp: bf16 bits = round(x * 128/ln2 + B). B centered for
    # min variation; the shared multiplicative bias cancels in softmax.
    EXP_A = 128.0 / float(np.log(2.0))
    EXP_B = 16246.8
    DVE_NI = (2, 5, 8, 11)  # key-chunks whose exp runs on VectorE

    nc = bacc.Bacc("TRN2", target_bir_lowering=False, debug=False)

    # ---- I/O (layouts chosen so every DMA is contiguous) ----
    xt_m = nc.dram_tensor("xt_m", [128, 4, EC // 2, 2, 512], FP8,
                          kind="ExternalInput").ap()
    mbt = nc.dram_tensor("mbt", [HH * DK, M], FP16, kind="ExternalInput").ap()
    xt_e = nc.dram_tensor("xt_e", [128, 3, EC, 512], FP16, kind="ExternalInput").ap()
    xt_e8 = nc.dram_tensor("xt_e8", [128, 3, EC // 2, 2, 512], FP8,
                           kind="ExternalInput").ap()
    ebt = nc.dram_tensor("ebt", [HH * DK, NLOC], FP16, kind="ExternalInput").ap()
    wq = nc.dram_tensor("wq", [128, PP, EC // 2, 2, 128], FP8,
                        kind="ExternalInput").ap()
    wk = nc.dram_tensor("wk", [128, PP, EC // 2, 2, 128], FP8,
                        kind="ExternalInput").ap()
    wv = nc.dram_tensor("wv", [128, EC, TT], FP16, kind="ExternalInput").ap()
    bq = nc.dram_tensor("bq", [128, PP], F32, kind="ExternalInput").ap()
    out_p = nc.dram_tensor("out_p", [HH, M, DK + 1], F32, kind="ExternalOutput").ap()

    with tile.TileContext(nc) as tc, ExitStack() as ctx:
        persist = ctx.enter_context(tc.tile_pool(name="persist", bufs=1))
        qpt_pool = ctx.enter_context(tc.tile_pool(name="qpt", bufs=5))
        attn_pool = ctx.enter_context(tc.tile_pool(name="attn", bufs=24))
        osb_pool = ctx.enter_context(tc.tile_pool(name="osb", bufs=6))
        # PSUM: scores 3 x [128,1024] (6 banks) + one shared pool for the
        # proj [128,512] and av [128,4,65] accumulators (2 banks) = 8.
        # 3 score bufs decouple PE from ACT/DVE exp-latency jitter.
        work_ps = ctx.enter_context(tc.tile_pool(name="work_ps", bufs=2, space="PSUM"))
        sc_ps = ctx.enter_context(tc.tile_pool(name="sc_ps", bufs=3, space="PSUM"))
        proj_ps = work_ps
        av_ps = work_ps

        # K'T per head [128, NLOC]: rows = k-dims | eb-dims (parity layout:
        # even head k at partitions 0:64, odd head k at 64:128 - avoids any
        # cross-partition copies; scores only need a consistent d' order)
        kpt = [persist.tile([128, NLOC], FP16, tag=f"kpt{h}", name=f"kpt{h}")
               for h in range(HH)]
        # V per key chunk [128, HH, DK+1] bf16, ones col at [., ., DK]
        vsb = [persist.tile([128, HH, DK + 1], BF16, tag=f"v{ni}", name=f"v{ni}")
               for ni in range(NI)]
        bq_sb = persist.tile([128, PP], F32, tag="bq")
        wk_big = persist.tile([128, PP, EC // 2, 2, 128], FP8, tag="wk")
        wq_big = persist.tile([128, PP, EC // 2, 2, 128], FP8, tag="wq")
        wv_big = persist.tile([128, EC, TT], FP16, tag="wv")
        xte_big = persist.tile([128, 3, EC, 512], FP16, tag="xte")
        xte8_big = persist.tile([128, 3, EC // 2, 2, 512], FP8, tag="xte8")
        xtm_big = persist.tile([128, 4, EC // 2, 2, 512], FP8, tag="xtm")
        q0_0 = qpt_pool.tile([128, M], FP16, tag="qpt", name="qpt0")
        q1_0 = qpt_pool.tile([128, M], FP16, tag="qpt", name="qpt1")

        # input DMAs: need-ordered across the two HWDGE rings (sync=SP,
        # scalar=Act) plus gpsimd SWDGE rows.
        # measured ring rates under contention: sync(SP)~108GB/s,
        # scalar(Act)~69GB/s, gpsimd(SWDGE)~126GB/s. Critical chains are
        # spread by deadline: first-scores needs wk8+xte8_0+ebt_h0 (SP),
        # wq8 (Act), mbt_h0+xtm8_0/1 (SW) by ~15us; kt thirds 1/2 and the
        # half-1 quarters follow; bulk fp16 xte/wv (V-proj, ~35us) last.
        nc.scalar.dma_start(bq_sb[:], bq)
        nc.sync.dma_start(wk_big[:, 0], wk[:, 0])
        nc.scalar.dma_start(wq_big[:, 0], wq[:, 0])
        nc.gpsimd.dma_start(q0_0[64:128, :], mbt[0:DK, :])
        nc.sync.dma_start(xte8_big[:, 0], xt_e8[:, 0])
        nc.gpsimd.dma_start(xtm_big[:, 0], xt_m[:, 0])
        nc.scalar.dma_start(xte8_big[:, 2], xt_e8[:, 2])
        nc.sync.dma_start(kpt[0][64:128, :], ebt[0:DK, :])
        nc.gpsimd.dma_start(xtm_big[:, 1], xt_m[:, 1])
        nc.sync.dma_start(xte8_big[:, 1], xt_e8[:, 1])
        nc.gpsimd.dma_start(q1_0[0:64, :], mbt[DK:2 * DK, :])
        nc.gpsimd.dma_start(kpt[1][0:64, :], ebt[DK:2 * DK, :])
        # remainder
        nc.scalar.dma_start(xtm_big[:, 3], xt_m[:, 3])
        nc.gpsimd.dma_start(xtm_big[:, 2], xt_m[:, 2])
        nc.scalar.dma_start(wk_big[:, 1:PP], wk[:, 1:PP])
        nc.gpsimd.dma_start(wv_big[:], wv[:])
        nc.sync.dma_start(xte_big[:, 0], xt_e[:, 0])
        nc.sync.dma_start(xte_big[:, 1], xt_e[:, 1])
        nc.gpsimd.dma_start(xte_big[:, 2], xt_e[:, 2])
        nc.sync.dma_start(wq_big[:, 1:PP], wq[:, 1:PP])

        # PE warm-up: ~10 dummy matmuls on zeroed tiles run during the
        # initial DMA wait (PE is otherwise idle until ~12.6us) so the
        # HAM clock-gate flips to 8/8 before the real ramp matmuls --
        # those otherwise run at 1.2GHz for the first ~3.4us-busy window
        # (12.2us throttle_active measured, ~5.6us extra MM latency).
        warm_w = persist.tile([128, 128], FP16, tag="warm_w")
        warm_x = persist.tile([128, 512], FP16, tag="warm_x")
        nc.vector.memset(warm_w[:], 0.0)
        nc.vector.memset(warm_x[:], 0.0)
        wps = proj_ps.tile([128, 512], F32, tag="work", name="warm_ps")
        for wi in range(10):
            nc.tensor.matmul(wps[:], lhsT=warm_w[:], rhs=warm_x[:],
                             start=(wi == 0), stop=(wi == 9))

        def emit_scores_mj(ps, h, qt, ni, half, mj):
            mo = half * 1024
            nc.tensor.matmul(
                ps[:, mj * 512:(mj + 1) * 512],
                lhsT=kpt[h][:, ni * 128:(ni + 1) * 128],
                rhs=qt[:, mo + mj * 512:mo + (mj + 1) * 512],
                start=True, stop=True)

        def emit_exp(ps, h, ni, half, at):
            mo = half * 1024
            if ni in DVE_NI:
                nc.vector.tensor_scalar(
                    at[:, mo:mo + 1024].bitcast(U16), ps[:],
                    EXP_A, EXP_B, op0=Mult, op1=Add)
            else:
                nc.scalar.activation(at[:, mo:mo + 1024], ps[:], Exp)

        def emit_scores_exp_half(h, qt, ni, half, at, pre=None):
            """scoresT half [128 keys, 1024 queries] + exp into attnT.
            exp runs on ScalarE (exact) or VectorE (Schraudolph bits)
            depending on the key-chunk, so the two engines drain scores
            psum concurrently."""
            ps = pre if pre is not None else sc_ps.tile(
                [128, 1024], F32, tag="sc", name="sc_ps_t")
            for mj in ((1,) if pre is not None else (0, 1)):
                emit_scores_mj(ps, h, qt, ni, half, mj)
            emit_exp(ps, h, ni, half, at)

        def emit_av(h, attns, g4):
            """out chunks for 4 query-blocks mi in [4*g4, 4*g4+4); the
            bf16 attnT chunk is the stationary operand (fast weight
            load). One grouped copy + DMA per 4 blocks."""
            ps = av_ps.tile([128, 4, DK + 1], F32, tag="work", name="av_ps_t")
            for j in range(4):
                mi = 4 * g4 + j
                for ni in range(NI):
                    nc.tensor.matmul(
                        ps[:, j, :], lhsT=attns[ni][:, mi * 128:(mi + 1) * 128],
                        rhs=vsb[ni][:, h, :],
                        start=(ni == 0), stop=(ni == NI - 1))
            ot = osb_pool.tile([128, 4, DK + 1], F32, tag="osb", name="osb_t")
            nc.vector.tensor_copy(ot[:], ps[:])
            m0 = 4 * g4 * 128
            nc.sync.dma_start(
                out_p[h, m0:m0 + 512, :].rearrange("(j p) d -> p j d", p=128),
                ot[:])

        # ---- unit-queue scheduler ----
        units = deque()
        qts = {0: q0_0, 1: q1_0}
        pieces = {0: set()}  # pair -> done piece ids (k0..k2, q0..q3)

        def emit_qt_quarter(p, mh, q0, q1):
            ps = proj_ps.tile([128, 512], F32, tag="work", name="proj_qt")
            for e2 in range(EC // 2):
                nc.tensor.matmul(ps[:], lhsT=wq_big[:, p, e2],
                                 rhs=xtm_big[:, mh, e2],
                                 start=(e2 == 0), stop=(e2 == EC // 2 - 1),
                                 perf_mode=DoubleRow)
            mo = mh * 512
            nc.vector.tensor_scalar(
                q0[0:64, mo:mo + 512], ps[0:64, :], 0.015625,
                bq_sb[0:64, p:p + 1], op0=Mult, op1=Add)
            nc.vector.tensor_scalar(
                q1[64:128, mo:mo + 512], ps[64:128, :], 0.015625,
                bq_sb[64:128, p:p + 1], op0=Mult, op1=Add)

        def qt_unit(p, mh):
            def f():
                pieces.setdefault(p, set()).add(f"q{mh}")
                q0, q1 = qts.get(2 * p), qts.get(2 * p + 1)
                if q0 is None:
                    q0 = qpt_pool.tile([128, M], FP16, tag="qpt", name=f"qpt{2*p}")
                    q1 = qpt_pool.tile([128, M], FP16, tag="qpt", name=f"qpt{2*p+1}")
                    h0, h1 = 2 * p, 2 * p + 1
                    nc.gpsimd.dma_start(q0[64:128, :], mbt[h0 * DK:(h0 + 1) * DK, :])
                    nc.gpsimd.dma_start(q1[0:64, :], mbt[h1 * DK:(h1 + 1) * DK, :])
                    qts[2 * p], qts[2 * p + 1] = q0, q1
                emit_qt_quarter(p, mh, q0, q1)
            return (1.35, f)

        def emit_kt_third(p, t):
            h0, h1 = 2 * p, 2 * p + 1
            lo = t * 512
            ps = proj_ps.tile([128, 512], F32, tag="work", name="proj_kt")
            for e2 in range(EC // 2):
                nc.tensor.matmul(ps[:], lhsT=wk_big[:, p, e2],
                                 rhs=xte8_big[:, t, e2],
                                 start=(e2 == 0), stop=(e2 == EC // 2 - 1),
                                 perf_mode=DoubleRow)
            nc.vector.tensor_scalar(kpt[h0][0:64, lo:lo + 512], ps[0:64, :],
                                    0.125, None, op0=Mult)
            nc.vector.tensor_scalar(kpt[h1][64:128, lo:lo + 512], ps[64:128, :],
                                    0.125, None, op0=Mult)
            if t == 0 and p > 0:
                nc.gpsimd.dma_start(kpt[h0][64:128, :], ebt[h0 * DK:(h0 + 1) * DK, :])
                nc.gpsimd.dma_start(kpt[h1][0:64, :], ebt[h1 * DK:(h1 + 1) * DK, :])

        def kt_unit(p, t):
            def f():
                pieces.setdefault(p, set()).add(f"k{t}")
                emit_kt_third(p, t)
            return (1.35, f)

        def v_unit(ni):
            def f():
                ps = proj_ps.tile([128, TT], F32, tag="work", name="proj_v")
                t, off = divmod(ni, 4)
                for ec in range(EC):
                    nc.tensor.matmul(
                        ps[:], lhsT=xte_big[:, t, ec, off * 128:(off + 1) * 128],
                        rhs=wv_big[:, ec, :], start=(ec == 0), stop=(ec == EC - 1))
                nc.vector.tensor_copy(
                    vsb[ni][:, :, 0:DK], ps[:].rearrange("p (h d) -> p h d", d=DK))
                nc.vector.memset(vsb[ni][:, :, DK], 1.0)
            return (1.0, f)

        def av_unit(h, attns, g4):
            def f():
                emit_av(h, attns, g4)
            return (1.6, f)

        def pump(budget):
            while units and budget > 0:
                c, f = units.popleft()
                f()
                budget -= c

        # minimal head-0 critical path up front: kt third0 + qt q0/q1
        emit_kt_third(0, 0)
        pieces[0].add("k0")
        qt_unit(0, 0)[1]()
        qt_unit(0, 1)[1]()
        units.append(kt_unit(0, 2))
        units.append(kt_unit(0, 1))
        units.append(qt_unit(0, 2))
        units.append(qt_unit(0, 3))
        for ni in range(NI):
            units.append(v_unit(ni))

        def need(p, ni, half):
            req = {f"k{ni // 4}", f"q{2 * half}", f"q{2 * half + 1}"}
            while not req <= pieces.get(p, set()):
                c, f = units.popleft()
                f()

        slot = 0
        for h in range(HH):
            p = h // 2
            if h % 2 == 1 and p + 1 <= PP - 1:
                # next pair's projections jump the queue (front) so the
                # even-head boundary never force-drains a big batch
                for mh in range(3, -1, -1):
                    units.appendleft(qt_unit(p + 1, mh))
                for t in range(2, -1, -1):
                    units.appendleft(kt_unit(p + 1, t))
            attns = [attn_pool.tile([128, M], BF16, tag="attn",
                                    name=f"attn_{h}_{ni}") for ni in range(NI)]
            for half in range(2):
                # h0-half0: key-third 2 (Act ring) lands before third 1
                # (SP ring) -- consume chunks in DMA-arrival order
                ni_order = ([0, 1, 2, 3, 8, 9, 10, 11, 4, 5, 6, 7]
                            if (h == 0 and half == 0) else range(NI))
                for ni in ni_order:
                    need(p, ni, half)
                    emit_scores_exp_half(h, qts[h], ni, half, attns[ni])
                    # no pumping on the first 2 chunks after a half
                    # boundary: the queued av group would head-of-line
                    # block PE on the previous half's last exp tile
                    if slot < 12:
                        b = 0.0
                    elif slot < 24:
                        b = 2.5
                    elif ni < 2:
                        b = 0.0
                    else:
                        b = 1.1
                    pump(b)
                    slot += 1
                # av groups for mi chunks covered by this half can go
                # into the queue now (g4<2 reads attnT cols 0:1024 only)
                gs = range(2) if half == 0 else range(2, 4)
                for g4 in gs:
                    units.append(av_unit(h, attns, g4))
            qts[h] = None  # release the qpt slot
        while units:
            c, f = units.popleft()
            f()

    nc.compile()
    return nc


def _get_nc():
    if "nc" not in _CACHE:
        _CACHE["nc"] = _build()
    return _CACHE["nc"]


def kernel(**inputs):
    global LAST_EXEC_NS, LAST_TRACE_DIR
    from concourse.bass_utils import run_bass_kernel_spmd

    ehr = np.asarray(inputs["ehr_embeddings"], dtype=np.float32)
    mi = np.asarray(inputs["missing_indices"]).astype(np.int64)
    ei = np.asarray(inputs["exist_indices"]).astype(np.int64)
    Wq = np.asarray(inputs["Wq"], dtype=np.float32)
    Wk = np.asarray(inputs["Wk"], dtype=np.float32)
    Wv = np.asarray(inputs["Wv"], dtype=np.float32)
    bq = np.asarray(inputs["bq"], dtype=np.float32)
    bv = np.asarray(inputs["bv"], dtype=np.float32)
    cooc = np.asarray(inputs["cooc_bias"], dtype=np.float32)

    scale = 1.0 / np.sqrt(np.float32(DK))

    def fold(a):  # [E, F] -> [128, EC, F]
        return a.reshape(EC, 128, a.shape[1]).transpose(1, 0, 2)

    def wfold(a):  # [E, TT] -> [128, PP, EC, 128] (pair-col major)
        return np.ascontiguousarray(
            fold(a).reshape(128, EC, PP, 128).transpose(0, 2, 1, 3))

    import ml_dtypes
    F8NP = ml_dtypes.float8_e4m3

    missing_emb = ehr[mi]                       # [M, E]
    # Q-proj operands in fp8 e4m3 (DoubleRow): Wq pre-scaled by 8 so its
    # values sit in fp8 normal range; the kernel rescales psum by 1/64.
    xt_m = np.ascontiguousarray(
        fold(missing_emb.T)
        .reshape(128, EC, 4, 512).transpose(0, 2, 1, 3)
        .reshape(128, 4, EC // 2, 2, 512)).astype(F8NP)
    wq_all = (Wq * 8.0).astype(np.float32)
    wk_all = (Wk * 8.0).astype(np.float32)
    wv_all = Wv.astype(np.float16)
    mbt_all = cooc[:, mi, :].transpose(0, 2, 1).reshape(H * DK, M).astype(np.float16)
    bq_all = (bq * scale).astype(np.float32)

    in_maps = []
    for c in range(CORES):
        hg, ns = c // NSHARDS, c % NSHARDS
        hsl = slice(hg * TT, (hg + 1) * TT)
        eic = ei[ns * NLOC:(ns + 1) * NLOC]
        xte_f32 = fold(ehr[eic].T)                   # [128, EC, NLOC]
        xte_t = xte_f32.reshape(128, EC, 3, 512).transpose(0, 2, 1, 3)
        xt_e = np.ascontiguousarray(xte_t).astype(np.float16)
        xt_e8 = np.ascontiguousarray(
            xte_t.reshape(128, 3, EC // 2, 2, 512)).astype(F8NP)
        ebt = np.ascontiguousarray(
            cooc[hg * HH:(hg + 1) * HH, eic, :].transpose(0, 2, 1)
            .reshape(HH * DK, NLOC).astype(np.float16))
        in_maps.append({
            "xt_m": xt_m,
            "mbt": np.ascontiguousarray(mbt_all[hsl]),
            "xt_e": xt_e, "xt_e8": xt_e8, "ebt": ebt,
            "wq": np.ascontiguousarray(
                wfold(wq_all[:, hsl]).reshape(128, PP, EC // 2, 2, 128)
            ).astype(F8NP),
            "wk": np.ascontiguousarray(
                wfold(wk_all[:, hsl]).reshape(128, PP, EC // 2, 2, 128)
            ).astype(F8NP),
            "wv": np.ascontiguousarray(fold(wv_all[:, hsl])),
            "bq": np.ascontiguousarray(bq_all[hsl].reshape(PP, 128).T),
        })

    nc = _get_nc()
    kwargs = {}
    if os.environ.get("KERNEL_TRACE") == "1":
        import tempfile
        LAST_TRACE_DIR = tempfile.mkdtemp(prefix="kern_trace_")
        kwargs = {"trace": True, "tmpdir": LAST_TRACE_DIR}
        try:
            import ntff_shim
            ntff_shim.install()
        except ImportError:
            pass
    res = run_bass_kernel_spmd(nc, in_maps, list(range(CORES)), **kwargs)
    LAST_EXEC_NS = res.exec_time_ns

    # ---- host combine (exact softmax across the 4 key shards) ----
    num = np.zeros((H, M, DK), dtype=np.float64)
    den = np.zeros((H, M), dtype=np.float64)
    for c in range(CORES):
        hg = c // NSHARDS
        op = res.results[c]["out_p"].astype(np.float64)  # [HH, M, DK+1]
        num[hg * HH:(hg + 1) * HH] += op[:, :, :DK]
        den[hg * HH:(hg + 1) * HH] += op[:, :, DK]
    out = num / den[:, :, None]                          # [H, M, DK]
    out = out.transpose(1, 0, 2).reshape(M, TOTAL) + bv.astype(np.float64)
    result = ehr.copy()
    result[mi] = out.astype(np.float32)
    return result

